# revision 6
# baseline (speedup 1.0000x reference)
"""Trainium2 Bass kernel for nn_MetadataEncoder (embedding_lookup).

Math: out = lrelu(concat(emb, cont) @ W.T + b), emb = 21 table lookups of
16 dims each, cont = lrelu(x_cont @ W_c.T + b_c) (8 dims).

Strategy (v2): on-chip gather + PE matmul.
  The raw embedding tables are tiny (21*1000*16 f32 = 1.3MB) and are held
  in SBUF TRANSPOSED: partition p = 16*g + d holds feature dim d of table
  t(bank, g); the free dim indexes the 1000 categories. One ap_gather
  (GPSIMD, 8 Q7 cores, each core serving its own 16 partitions with its own
  index stream) gathers 8 tables' features for a chunk of rows in a single
  instruction. 3 banks cover 21 tables + a quantized cont-branch table
  (4096 levels, includes lrelu and a constant-1 feature for the bias).
  The gathered activations X land feature-major ([features, rows]) so the
  [344->32] MLP is 3 accumulated fp32r matmuls (full PE rate at N=512).
  LeakyReLU epilogue on DVE, output written [32, rows] and transposed on
  the host.

Per core (62500 rows): Pool ~3*62500 gathered idx, PE ~4.7M psum elems,
DMA only idx in (~1.5MB) and out (~8MB). No HBM random access at all.
"""
import numpy as np

import concourse.bacc as bacc
import concourse.mybir as mybir
import concourse.tile as tile
from concourse.bass_utils import run_bass_kernel_spmd

NUM_TABLES = 21
NUM_CATS = 1000
EMB_DIM = 16
B = 500000
OUT_DIM = 32
NEG_SLOPE = 0.01
QCONT = 4096                     # cont-branch quantization levels
N_CORES = 8
SHARD = B // N_CORES             # 62500
CHUNK = 2048                     # rows per chunk (one ap_gather per bank)
NCHUNK = 31                      # 31*2048 = 63488 >= 62500
SHARD_PAD = NCHUNK * CHUNK
NSUB = CHUNK // 512              # matmul subtiles per chunk
IDXW = 3 * (CHUNK // 16)         # idx columns per chunk (3 banks)

_cache = {}


def _build(reps=1):
    nc = bacc.Bacc("TRN2", target_bir_lowering=False)
    f32, f32r, i16 = mybir.dt.float32, mybir.dt.float32r, mybir.dt.int16

    tabs_d = nc.dram_tensor("tabs", [128, 6144], f32, kind="ExternalInput")
    wts_d = nc.dram_tensor("wts", [384, OUT_DIM], f32, kind="ExternalInput")
    idx_d = nc.dram_tensor("idx", [128, NCHUNK * IDXW], i16, kind="ExternalInput")
    out_d = nc.dram_tensor("out", [OUT_DIM, SHARD_PAD], f32, kind="ExternalOutput")

    LR = mybir.AluOpType
    with tile.TileContext(nc) as tc:
        with tc.tile_pool(name="const", bufs=1) as constp, \
             tc.tile_pool(name="idxp", bufs=3) as idxp, \
             tc.tile_pool(name="xp", bufs=2) as xp, \
             tc.psum_pool(name="pp", bufs=8) as pp, \
             tc.tile_pool(name="outp", bufs=2) as outp:
            tabs_sb = constp.tile([128, 6144], f32, tag="tabs")
            nc.sync.dma_start(tabs_sb[:], tabs_d[:])
            w_ld = constp.tile([128, 3 * OUT_DIM], f32, tag="wts_ld")
            for bk in range(3):
                nc.sync.dma_start(w_ld[:, OUT_DIM * bk:OUT_DIM * (bk + 1)],
                                  wts_d[128 * bk:128 * (bk + 1), :])
            w_sb = w_ld
            for rep in range(reps):
                for k in range(NCHUNK):
                    idx_sb = idxp.tile([128, IDXW], i16, tag="idx")
                    nc.sync.dma_start(idx_sb[:], idx_d[:, IDXW * k:IDXW * (k + 1)])
                    iw = CHUNK // 16
                    xs = []
                    for bk in range(3):
                        x = xp.tile([128, CHUNK], f32, tag=f"x{bk}")
                        if bk < 2:
                            nc.gpsimd.ap_gather(
                                x[:],
                                tabs_sb[:, 1024 * bk:1024 * bk + NUM_CATS]
                                ,
                                idx_sb[:, iw * bk:iw * (bk + 1)],
                                channels=128, num_elems=NUM_CATS, d=1,
                                num_idxs=CHUNK)
                        else:
                            nc.gpsimd.ap_gather(
                                x[0:96, :],
                                tabs_sb[0:96, 2048:2048 + QCONT],
                                idx_sb[0:96, iw * 2:iw * 3],
                                channels=96, num_elems=QCONT, d=1,
                                num_idxs=CHUNK)
                        xs.append(x)
                    ob = outp.tile([OUT_DIM, CHUNK], f32, tag="ob")
                    for j in range(NSUB):
                        sl = slice(512 * j, 512 * (j + 1))
                        ps = pp.tile([OUT_DIM, 512], f32, tag="ps")
                        nc.tensor.matmul(
                            ps[:], w_sb[:, 0:OUT_DIM],
                            xs[0][:, sl], start=True, stop=False)
                        nc.tensor.matmul(
                            ps[:], w_sb[:, OUT_DIM:2 * OUT_DIM],
                            xs[1][:, sl], start=False, stop=False)
                        nc.tensor.matmul(
                            ps[:], w_sb[0:96, 2 * OUT_DIM:3 * OUT_DIM],
                            xs[2][0:96, sl], start=False, stop=True)
                        nc.vector.tensor_scalar(ob[:, sl], ps[:], NEG_SLOPE, None,
                                                LR.mult)
                        nc.vector.tensor_tensor(ob[:, sl], ob[:, sl], ps[:], LR.max)
                    nc.sync.dma_start(out_d[:, CHUNK * k:CHUNK * (k + 1)], ob[:])
    nc.compile()
    return nc


def _prep_consts(tables, W_c, b_c, W, b):
    """Host fold: transposed tables [128, 6144] + weight blocks [384, 32]."""
    t = np.asarray(tables, np.float32)                       # [21, 1000, 16]
    tabs = np.zeros((128, 6144), np.float32)
    tabs[:, 0:NUM_CATS] = t[0:8].transpose(0, 2, 1).reshape(128, NUM_CATS)
    tabs[:, 1024:1024 + NUM_CATS] = t[8:16].transpose(0, 2, 1).reshape(128, NUM_CATS)
    tabs[0:80, 2048:2048 + NUM_CATS] = t[16:21].transpose(0, 2, 1).reshape(80, NUM_CATS)
    xq = (np.arange(QCONT, dtype=np.float64) + 0.5) / QCONT
    v = xq[:, None] * np.asarray(W_c, np.float64)[None, :, 0] \
        + np.asarray(b_c, np.float64)[None, :]               # [Q, 8]
    v = np.where(v >= 0, v, NEG_SLOPE * v)
    tabs[80:88, 2048:2048 + QCONT] = v.T.astype(np.float32)
    tabs[88, 2048:2048 + QCONT] = 1.0                        # bias feature
    Wf = np.asarray(W, np.float32)                           # [32, 344]
    wts = np.zeros((384, OUT_DIM), np.float32)
    wts[0:128] = Wf[:, 0:128].T
    wts[128:256] = Wf[:, 128:256].T
    wts[256:336] = Wf[:, 256:336].T
    wts[336:344] = Wf[:, 336:344].T
    wts[344] = np.asarray(b, np.float32)
    return tabs, wts


def _prep_idx(x_cat_shard, x_cont_shard):
    """[128, NCHUNK*IDXW] int16 for one core.

    Stream s = 8*bank + group: s<21 -> table s, s=21 -> quantized cont,
    s=22,23 -> 0. Index i of chunk k lands at partition 16g + (i%16),
    column k*IDXW + bank*(CHUNK//16) + i//16.
    """
    n = x_cat_shard.shape[0]
    arr = np.zeros((24, SHARD_PAD), np.int16)
    arr[:NUM_TABLES, :n] = np.asarray(x_cat_shard, np.int64).T.astype(np.int16)
    q = np.clip((np.asarray(x_cont_shard)[:, 0] * QCONT).astype(np.int32),
                0, QCONT - 1)
    arr[NUM_TABLES, :n] = q.astype(np.int16)
    a = arr.reshape(3, 8, NCHUNK, CHUNK // 16, 16)           # (b, g, k, v, w)
    return np.ascontiguousarray(a.transpose(1, 4, 2, 0, 3)).reshape(
        128, NCHUNK * IDXW)


def kernel(x_cat, x_cont, tables, W_c, b_c, W, b):
    if "nc" not in _cache:
        _cache["nc"] = _build()
    nc = _cache["nc"]
    tabs, wts = _prep_consts(tables, W_c, b_c, W, b)
    x_cat = np.asarray(x_cat)
    x_cont = np.asarray(x_cont)
    in_maps = []
    for c in range(N_CORES):
        sl = slice(c * SHARD, (c + 1) * SHARD)
        in_maps.append({
            "tabs": tabs,
            "wts": wts,
            "idx": _prep_idx(x_cat[sl], x_cont[sl]),
        })
    res = run_bass_kernel_spmd(nc, in_maps, core_ids=list(range(N_CORES)))
    outs = []
    for c in range(N_CORES):
        o = np.asarray(res.results[c]["out"])                # [32, SHARD_PAD]
        outs.append(o.T[:SHARD])
    return np.ascontiguousarray(np.concatenate(outs, axis=0))


# revision 8
# speedup vs baseline: 1.3003x; 1.3003x over previous
"""Trainium2 Bass kernel for nn_MetadataEncoder (embedding_lookup).

Math: out = lrelu(concat(emb, cont) @ W.T + b), emb = 21 table lookups.
The MLP is linear, so fold W into the tables on the host:
    P_t = tables[t] @ W[:, 16t:16t+16].T            -> [1000, 32] each
and fold the continuous branch + bias into a quantized 22nd table:
    C[q] = lrelu(x_q @ W_c.T + b_c) @ W[:, 336:344].T + b
Then per row r: out[r] = lrelu( sum_t P_t[x_cat[r,t]] + C[quant(x_cont[r])] ).

v4 device strategy: SWDGE dma_gather with 64-BYTE descriptors. The stock
bass wrapper forces 256B elements, but the Q7 generator and decode only
require 256B granularity for (a) the table row STRIDE and (b) transpose
mode — a non-transpose gather with elem_size=32 bf16 (64B) and a 256B row
stride is valid and measured ~1.7ns/descriptor vs ~2.7ns for 256B ones
(SWDGE is descriptor-count-bound, so smaller payloads win).

Per core: 62592 rows x 22 streams = 1.38M descriptors, gathered in blocks
of 8192 rows across 4 SWDGE queues; bf16 DVE accumulate (2x mode) + lrelu;
row-wrapped output [128, 489*32] bf16 (row r at partition r%128).
idx is pre-replicated to all 128 partitions on the host (int16).
"""
import numpy as np
import ml_dtypes

import concourse.bacc as bacc
import concourse.mybir as mybir
import concourse.tile as tile
from concourse.bass_utils import run_bass_kernel_spmd

NUM_TABLES = 21
NUM_CATS = 1000
EMB_DIM = 16
B = 500000
OUT_DIM = 32
NEG_SLOPE = 0.01
QCONT = 4096                      # cont-branch quantization levels
N_CORES = 8
SHARD = B // N_CORES              # 62500
SHARD_PAD = 62592                 # 489 * 128
NS = 22                           # payload streams per row
TAB_ROWS = NUM_TABLES * NUM_CATS + QCONT   # 25096
ES = 128                          # table row stride in bf16 elems (256B)
BLK = 8192                        # rows per block
NQ = 4

_cache = {}


def _blocks():
    blks, r = [], 0
    while r < SHARD_PAD:
        nb = min(BLK, SHARD_PAD - r)
        blks.append((r, nb))
        r += nb
    return blks


def _dma_gather64(nc, out_ap, in_ap, idxs_ap, num_idxs, queue_num):
    """Non-transpose SWDGE gather with 64B elems (bypasses the 256B-elem
    wrapper assert; the ucode only needs the row STRIDE 256B-aligned)."""
    g = nc.gpsimd
    g._assert_queue_num(queue_num)
    assert idxs_ap.dtype == mybir.dt.int16
    _in_ap = g.lower_ap_dma(in_ap, for_custom_bir_dma=True)
    _idxs_ap = g.lower_ap(idxs_ap)
    _out_ap = g.lower_ap(out_ap)
    return g.add_instruction(
        mybir.InstDMAGatherAnt(
            name=nc.get_next_instruction_name(),
            ins=[*_in_ap, _idxs_ap, g.lower_val_access(g.to_reg(num_idxs))],
            outs=[_out_ap],
            transpose=False,
            num_idxs=num_idxs,
            elem_size=OUT_DIM,
            stride_bytes_256=ES * 2 // 256,
            gen_mode=0,
            single_packet=False,
            queue_num=queue_num,
            sbuf_tokens_per_rank=0,
            sbuf_free_dim_per_rank=0,
            sbuf_free_dim_pad_per_rank=0,
            sbuf_byte_offset=0,
        ))


def _build(reps=1):
    nc = bacc.Bacc("TRN2", target_bir_lowering=False, num_swdge_queues=NQ)
    bf16, i16 = mybir.dt.bfloat16, mybir.dt.int16
    blks = _blocks()
    idx_cols = sum((nb // 16) * NS for _, nb in blks)

    tab_d = nc.dram_tensor("tab", [TAB_ROWS, ES], bf16, kind="ExternalInput")
    idx_d = nc.dram_tensor("idx", [128, idx_cols], i16, kind="ExternalInput")
    out_d = nc.dram_tensor("out", [128, (SHARD_PAD // 128) * OUT_DIM], bf16,
                           kind="ExternalOutput")

    LR = mybir.AluOpType
    with tile.TileContext(nc) as tc:
        with tc.tile_pool(name="idxp", bufs=2) as idxp, \
             tc.tile_pool(name="slabp", bufs=8) as slabp, \
             tc.tile_pool(name="accp", bufs=2) as accp, \
             tc.tile_pool(name="outp", bufs=2) as outp:
            for rep in range(reps):
                icol = 0
                for r0, nb in blks:
                    iw = nb // 16
                    nrow = nb // 128
                    idx_sb = idxp.tile([128, (BLK // 16) * NS], i16, tag="idx")
                    nc.sync.dma_start(idx_sb[:, 0:iw * NS],
                                      idx_d[:, icol:icol + iw * NS])
                    icol += iw * NS
                    acc = accp.tile([128, (BLK // 128) * OUT_DIM], bf16,
                                    tag="acc")
                    slabs = []
                    for t in range(NS):
                        slab = slabp.tile([128, (BLK // 128) * OUT_DIM], bf16,
                                          tag="slab")
                        _dma_gather64(
                            nc,
                            slab[:, 0:nrow * OUT_DIM].rearrange(
                                "p (n e) -> p n e", e=OUT_DIM),
                            tab_d[:, 0:OUT_DIM],
                            idx_sb[:, t * iw:(t + 1) * iw],
                            num_idxs=nb,
                            queue_num=t % NQ)
                        slabs.append(slab)
                        if t == 1:
                            nc.vector.tensor_tensor(
                                acc[:, 0:nrow * OUT_DIM],
                                slabs[0][:, 0:nrow * OUT_DIM],
                                slabs[1][:, 0:nrow * OUT_DIM], LR.add)
                        elif t > 1:
                            nc.vector.tensor_tensor(
                                acc[:, 0:nrow * OUT_DIM],
                                acc[:, 0:nrow * OUT_DIM],
                                slab[:, 0:nrow * OUT_DIM], LR.add)
                    ob = outp.tile([128, (BLK // 128) * OUT_DIM], bf16, tag="ob")
                    nc.vector.tensor_scalar(ob[:, 0:nrow * OUT_DIM],
                                            acc[:, 0:nrow * OUT_DIM],
                                            NEG_SLOPE, None, LR.mult)
                    nc.vector.tensor_tensor(ob[:, 0:nrow * OUT_DIM],
                                            ob[:, 0:nrow * OUT_DIM],
                                            acc[:, 0:nrow * OUT_DIM], LR.max)
                    nc.sync.dma_start(
                        out_d[:, (r0 // 128) * OUT_DIM:
                              (r0 // 128 + nrow) * OUT_DIM],
                        ob[:, 0:nrow * OUT_DIM])
    nc.compile()
    return nc


def _prep_tab(tables, W_c, b_c, W, b):
    """Folded bf16 table [TAB_ROWS, ES]; payload in first 32 cols."""
    td = np.asarray(tables, np.float64)
    Wd = np.asarray(W, np.float64)
    tab = np.zeros((TAB_ROWS, ES), np.float64)
    for t in range(NUM_TABLES):
        tab[NUM_CATS * t:NUM_CATS * (t + 1), :OUT_DIM] = \
            td[t] @ Wd[:, EMB_DIM * t:EMB_DIM * (t + 1)].T
    xq = (np.arange(QCONT, dtype=np.float64) + 0.5) / QCONT
    v = xq[:, None] * np.asarray(W_c, np.float64)[None, :, 0] \
        + np.asarray(b_c, np.float64)[None, :]
    v = np.where(v >= 0, v, NEG_SLOPE * v)
    tab[NUM_TABLES * NUM_CATS:, :OUT_DIM] = \
        v @ Wd[:, NUM_TABLES * EMB_DIM:].T + np.asarray(b, np.float64)[None, :]
    return tab.astype(ml_dtypes.bfloat16)


def _prep_idx(x_cat_shard, x_cont_shard):
    """[128, idx_cols] int16: per block, per table, wrapped-16 and
    replicated to all 8 partition groups."""
    n = x_cat_shard.shape[0]
    flat = np.zeros((SHARD_PAD, NS), np.int16)
    base = (np.arange(NUM_TABLES, dtype=np.int32) * NUM_CATS)[None, :]
    flat[:n, :NUM_TABLES] = (np.asarray(x_cat_shard, np.int32) + base).astype(
        np.int16)
    q = np.clip((np.asarray(x_cont_shard)[:, 0] * QCONT).astype(np.int32),
                0, QCONT - 1)
    flat[:n, NUM_TABLES] = (NUM_TABLES * NUM_CATS + q).astype(np.int16)
    cols = []
    for r0, nb in _blocks():
        blk = flat[r0:r0 + nb]                         # [nb, NS]
        w = blk.T.reshape(NS, nb // 16, 16)            # idx i -> (i%16, i//16)
        cols.append(np.ascontiguousarray(w.transpose(0, 2, 1))
                    .reshape(NS, 16, nb // 16)
                    .transpose(1, 0, 2).reshape(16, -1))
    one = np.concatenate(cols, axis=1)                 # [16, idx_cols]
    return np.tile(one, (8, 1))                        # replicate to 128


def kernel(x_cat, x_cont, tables, W_c, b_c, W, b):
    if "nc" not in _cache:
        _cache["nc"] = _build()
    nc = _cache["nc"]
    tab = _prep_tab(tables, W_c, b_c, W, b)
    x_cat = np.asarray(x_cat)
    x_cont = np.asarray(x_cont)
    in_maps = []
    for c in range(N_CORES):
        sl = slice(c * SHARD, (c + 1) * SHARD)
        in_maps.append({
            "tab": tab,
            "idx": _prep_idx(x_cat[sl], x_cont[sl]),
        })
    res = run_bass_kernel_spmd(nc, in_maps, core_ids=list(range(N_CORES)))
    outs = []
    for c in range(N_CORES):
        o = np.asarray(res.results[c]["out"]).astype(np.float32)
        o = o.reshape(128, SHARD_PAD // 128, OUT_DIM)
        o = o.transpose(1, 0, 2).reshape(SHARD_PAD, OUT_DIM)[:SHARD]
        outs.append(o)
    return np.ascontiguousarray(np.concatenate(outs, axis=0))
